# revision 1
# baseline (speedup 1.0000x reference)
"""Trainium2 Bass kernel for nn_Attention_5669356830982.

Computes attn = softmax((E @ W.T + b) @ h)[None, None, :] for
E:[32768,1024], W:[1024,1024], h:[1024], b:[1024] (all fp32).

Key algebraic rewrite: (E @ W.T + b) @ h == E @ (W.T @ h) + (b @ h), and the
scalar (b @ h) shift cancels inside softmax.  So the kernel computes
v = W.T @ h (tiny) and energies = E @ v (memory-bound GEMV over 128 MiB),
then a distributed softmax.

Distribution over 8 NeuronCores:
  - E is sharded by sequence: core i gets rows [4096*i, 4096*(i+1)).
  - W and h are replicated; each core computes the full v = W.T @ h locally
    with 16 accumulating PE matmuls (h replicated along the free dim so the
    result lands broadcast across all 128 partitions).  This costs +4 MiB of
    DMA per core but keeps the only collective off the critical path: in
    this runtime the FIRST collective of an execution pays a ~47 us ncfw
    cold-start, so a dummy warm-up AllGather is fired at t~0 (depends on
    nothing, output unread) and the single real collective - the 64 B
    softmax-stats AllGather at the tail - runs warm (~7 us).
  - energies shard: fused multiply+reduce (affine_mul_reduce) on VectorE.
  - Softmax: each core computes local max m_i and s_i = sum(exp(e - m_i)),
    AllGathers the (m_i, s_i) pairs, and normalizes locally:
    attn = exp(e - m_i) * exp(m_i - M) / sum_j s_j exp(m_j - M).

DMA layout: E tiles are [128, 4, 1024] with partition fastest within each
512-row block ("(t s p) h"), so each partition reads 4 full E rows (4 KiB
descriptors) and the energy column c = 4t+s maps to sequence index 128c + p,
which makes the final (PE-transposed) store 32 contiguous 512 B rows.
E-tile DMAs alternate between the SP and ACT HWDGE rings because descriptor
generation costs ~6 us per 2 MiB tile on one sequencer.

Self-contained: hardcodes all shapes; needs numpy + the concourse stack.
"""

import os

import numpy as np

# NTFF profiling of a subset of cores crashes this runtime; if a harness
# profiles the run, make sure all cores are captured.
os.environ.setdefault("BASS_PERFETTO_PROFILE_ALL_CORES", "1")

HIDDEN = 1024
SEQ = 32768
N_CORES = 8
S_SHARD = SEQ // N_CORES       # 4096 rows of E per core
P = 128
KT = HIDDEN // P               # 8 k-tiles of W
N_ETILES = 8                   # big E tiles per core
SEG = S_SHARD // (N_ETILES * P)  # 4 rows per partition per tile
NCOLS = N_ETILES * SEG         # 32 energy columns in SBUF

_CACHE = {}


def _build():
    import concourse.mybir as mybir
    import concourse.tile as tile
    from concourse import bacc
    from concourse import bass_isa
    from concourse.masks import make_identity

    f32 = mybir.dt.float32
    Alu = mybir.AluOpType
    Act = mybir.ActivationFunctionType
    Ax = mybir.AxisListType
    Red = bass_isa.ReduceOp

    nc = bacc.Bacc("TRN2", target_bir_lowering=False, debug=False,
                   num_devices=N_CORES)

    e_d = nc.dram_tensor("e", [S_SHARD, HIDDEN], f32, kind="ExternalInput").ap()
    w_d = nc.dram_tensor("w", [HIDDEN, HIDDEN], f32, kind="ExternalInput").ap()
    # h[j, k] = hidden[128*k + j] (host pre-transposed)
    h_d = nc.dram_tensor("h", [P, KT], f32, kind="ExternalInput").ap()
    o_d = nc.dram_tensor("attn", [S_SHARD], f32, kind="ExternalOutput").ap()

    rg = [list(range(N_CORES))]

    with tile.TileContext(nc) as tc:
        with (
            tc.tile_pool(name="epool", bufs=N_ETILES) as epool,
            tc.tile_pool(name="wpool", bufs=KT) as wpool,
            tc.tile_pool(name="small", bufs=1) as small,
            tc.tile_pool(name="psum", bufs=1, space="PSUM") as psum,
            tc.tile_pool(name="dram", bufs=1, space="DRAM") as dram,
        ):
            # ---- warm-up collective: absorbs the ~47us ncfw cold-start
            # concurrently with the DMA stream.  Output is never read.
            cc_w_in = dram.tile([1, 2], f32)
            cc_w_out = dram.tile([N_CORES, 2], f32)
            wz = small.tile([1, 2], f32, tag="wz")
            nc.vector.memset(wz[:], 0.0)
            nc.gpsimd.dma_start(cc_w_in[:], wz[:])
            nc.gpsimd.collective_compute(
                "AllGather", Alu.bypass, replica_groups=rg,
                ins=[cc_w_in[:].opt()], outs=[cc_w_out[:].opt()],
            )

            # ---------------- constants ----------------
            ident = small.tile([P, P], f32, tag="ident")
            make_identity(nc, ident[:])
            ones128 = small.tile([P, P], f32, tag="ones128")
            nc.vector.memset(ones128[:], 1.0)

            # Warm the ACT exp table early (~2.7us, overlaps the E stream).
            dummy = small.tile([1, 1], f32, tag="dummy")
            nc.vector.memset(dummy[:], 0.0)
            nc.scalar.activation(dummy[:], dummy[:], Act.Exp)

            # ---------------- v = W.T @ h (local, replicated) -----------
            # W k-tiles ride the ACT ring ahead of that ring's E tiles.
            h_sb = small.tile([P, KT], f32, tag="h_sb")
            w_dmas = [nc.scalar.dma_start(h_sb[:], h_d[:])]
            w_sb = []
            for k in range(KT):
                wt = wpool.tile([P, HIDDEN], f32, tag="w")
                w_dmas.append(
                    nc.scalar.dma_start(wt[:], w_d[k * P:(k + 1) * P, :]))
                w_sb.append(wt)

            # Pre-scale W rows by h (DVE fp32 2x mode) so every matmul uses
            # the constant all-ones stationary: one LDWEIGHTS total, and the
            # 16 accumulating matmuls stream back-to-back.
            for k in range(KT):
                nc.vector.tensor_scalar_mul(w_sb[k][:], w_sb[k][:],
                                            h_sb[:, k:k + 1])
            pvb = psum.tile([P, HIDDEN], f32, tag="pvb")
            for k in range(KT):
                for n in range(2):
                    nc.tensor.matmul(pvb[:, n * 512:(n + 1) * 512],
                                     lhsT=ones128[:],
                                     rhs=w_sb[k][:, n * 512:(n + 1) * 512],
                                     start=(k == 0), stop=(k == KT - 1))
            v_sb = small.tile([P, HIDDEN], f32, tag="v_sb")
            for n in range(2):  # bank-aligned PSUM reads
                nc.vector.tensor_copy(v_sb[:, n * 512:(n + 1) * 512],
                                      pvb[:, n * 512:(n + 1) * 512])

            # ---------------- energies = E @ v ----------------
            # Tile t, partition p, segment s holds E row 128*(4t+s) + p, so
            # energy column c = 4t+s maps to sequence index 128*c + p.
            e_view = e_d.rearrange("(t s p) h -> t p s h",
                                   t=N_ETILES, s=SEG, p=P)
            energies = small.tile([P, NCOLS], f32, tag="energies")
            scratch = small.tile([P, HIDDEN], f32, tag="scratch")
            from concourse.bass import _add_dep_helper
            for t in range(N_ETILES):
                et = epool.tile([P, SEG, HIDDEN], f32, tag="et")
                # All E tiles on the SP ring: a single HWDGE ring drains
                # FIFO, so tiles complete sequentially (~5.3us apart) and the
                # AMR chain pipelines tightly instead of all tiles finishing
                # together at the end of the stream.
                ed = nc.sync.dma_start(et[:], e_view[t])
                # Gate E traffic on W/h completion: W is only 4 MiB and v
                # (hence the whole DVE chain) depends on it, so letting the
                # 16.8 MB E stream interleave would delay v by ~15 us.
                for wd in w_dmas:
                    _add_dep_helper(ed.ins, wd.ins, sync=True,
                                    reason="E stream after W/h (v latency)")
                for s in range(SEG):
                    c = t * SEG + s
                    nc.vector.affine_mul_reduce(
                        out=scratch[:],
                        accum_out=energies[:, c:c + 1],
                        in0=et[:, s],
                        in1=v_sb[:],
                        scale=1.0,
                        bias=0.0,
                    )

            # ---------------- local softmax stats ----------------
            rowmax = small.tile([P, 1], f32, tag="rowmax")
            nc.vector.reduce_max(rowmax[:], energies[:], axis=Ax.X)
            m_all = small.tile([P, 1], f32, tag="m_all")
            nc.gpsimd.partition_all_reduce(m_all[:], rowmax[:], P, Red.max)
            negm_b = small.tile([P, 1], f32, tag="negm_b")
            nc.vector.tensor_scalar_mul(negm_b[:], m_all[:], -1.0)

            ex = small.tile([P, NCOLS], f32, tag="ex")
            rowsum = small.tile([P, 1], f32, tag="rowsum")
            nc.scalar.activation(ex[:], energies[:], Act.Exp,
                                 bias=negm_b[:], scale=1.0,
                                 accum_out=rowsum[:])
            s_all = small.tile([P, 1], f32, tag="s_all")
            nc.gpsimd.partition_all_reduce(s_all[:], rowsum[:], P, Red.add)

            stats = small.tile([1, 2], f32, tag="stats")
            nc.vector.tensor_copy(stats[:, 0:1], m_all[0:1, :])
            nc.vector.tensor_copy(stats[:, 1:2], s_all[0:1, :])

            # Small mid/tail DMAs ride SWDGE (gpsimd) so they never queue
            # behind E-tile descriptor generation on the HWDGE rings.
            cc_s_in = dram.tile([1, 2], f32)
            cc_s_out = dram.tile([N_CORES, 2], f32)
            nc.gpsimd.dma_start(cc_s_in[:], stats[:])
            nc.gpsimd.collective_compute(
                "AllGather", Alu.bypass, replica_groups=rg,
                ins=[cc_s_in[:].opt()], outs=[cc_s_out[:].opt()],
            )
            allst0 = small.tile([1, 2 * N_CORES], f32, tag="allst0")
            nc.gpsimd.dma_start(allst0[:],
                                cc_s_out[:].rearrange("r c -> (r c)")[None])
            allst_b = small.tile([P, N_CORES, 2], f32, tag="allst_b")
            nc.gpsimd.partition_broadcast(allst_b[:], allst0[:], P)

            # ---------------- global combine (on all partitions) --------
            m_vec = allst_b[:, :, 0]
            s_vec = allst_b[:, :, 1]
            Mg = small.tile([P, 1], f32, tag="Mg")
            nc.vector.reduce_max(Mg[:], m_vec, axis=Ax.X)
            d = small.tile([P, N_CORES], f32, tag="d")
            nc.vector.tensor_scalar(d[:], m_vec, Mg[:], None, op0=Alu.subtract)
            edv = small.tile([P, N_CORES], f32, tag="edv")
            nc.scalar.activation(edv[:], d[:], Act.Exp)
            wsum = small.tile([P, N_CORES], f32, tag="wsum")
            Sg = small.tile([P, 1], f32, tag="Sg")
            nc.vector.affine_mul_reduce(out=wsum[:], accum_out=Sg[:],
                                        in0=edv[:], in1=s_vec,
                                        scale=1.0, bias=0.0)
            # c0 = exp(m_loc - M) / Sg per partition; attn = ex * c0
            dm = small.tile([P, 1], f32, tag="dm")
            nc.vector.tensor_tensor(dm[:], m_all[:], Mg[:], op=Alu.subtract)
            edm = small.tile([P, 1], f32, tag="edm")
            nc.scalar.activation(edm[:], dm[:], Act.Exp)
            rS = small.tile([P, 1], f32, tag="rS")
            nc.vector.reciprocal(rS[:], Sg[:])
            c0_b = small.tile([P, 1], f32, tag="c0_b")
            nc.vector.tensor_tensor(c0_b[:], edm[:], rS[:], op=Alu.mult)

            attn_pad = small.tile([P, P], f32, tag="attn_pad")
            nc.vector.memset(attn_pad[:, NCOLS:], 0.0)
            nc.vector.tensor_scalar(attn_pad[:, :NCOLS], ex[:], c0_b[:], None,
                                    op0=Alu.mult)

            # Transpose [128, 32] (padded to 128) -> [32, 128] so the DRAM
            # write is 32 contiguous 512B rows: out[128*c + p] = attn_T[c, p].
            pat = psum.tile([P, P], f32, tag="pat")
            nc.tensor.transpose(pat[:], attn_pad[:], ident[:])
            attn_T = small.tile([NCOLS, P], f32, tag="attn_T")
            nc.vector.tensor_copy(attn_T[:], pat[:NCOLS, :])
            nc.sync.dma_start(o_d.rearrange("(c p) -> c p", p=P), attn_T[:])

    nc.compile()
    return nc


def _get_nc():
    if "nc" not in _CACHE:
        _CACHE["nc"] = _build()
    return _CACHE["nc"]


def _in_maps(hidden, E, W):
    h_t = np.ascontiguousarray(hidden.reshape(KT, P).T)  # h_t[j, k] = hidden[128k + j]
    W = np.ascontiguousarray(W)
    maps = []
    for i in range(N_CORES):
        maps.append({
            "e": np.ascontiguousarray(E[i * S_SHARD:(i + 1) * S_SHARD]),
            "w": W,
            "h": h_t,
        })
    return maps


def kernel(hidden, encoder_outputs, W, b):
    from concourse import bass_utils

    hidden = np.asarray(hidden, dtype=np.float32)
    E = np.ascontiguousarray(np.asarray(encoder_outputs, dtype=np.float32))
    W = np.asarray(W, dtype=np.float32)

    nc = _get_nc()
    res = bass_utils.run_bass_kernel_spmd(
        nc, _in_maps(hidden, E, W), core_ids=list(range(N_CORES)))
    attn = np.concatenate([res.results[i]["attn"] for i in range(N_CORES)])
    return attn.reshape(1, 1, SEQ).astype(np.float32)



# revision 8
# speedup vs baseline: 1.0315x; 1.0315x over previous
"""Trainium2 Bass kernel for nn_Attention_5669356830982.

Computes attn = softmax((E @ W.T + b) @ h)[None, None, :] for
E:[32768,1024], W:[1024,1024], h:[1024], b:[1024] (all fp32 in / fp32 out).

Algebraic rewrite: (E @ W.T + b) @ h == E @ (W.T @ h) + (b @ h); the scalar
(b @ h) shift cancels inside softmax.  So the kernel computes v = W.T @ h
(tiny) and energies = E @ v (memory-bound GEMV), then a distributed softmax.

v2 design (from the v1 trace: 132.6us, no engine >38% busy):
  - E and W are converted to bf16 on the host: halves the DMA stream
    (10.5 MiB/core total) and enables the DVE 2x_1p fast mode for the
    energy dot products.  Numerics: softmax is highly peaked (top-2
    energy gap ~5); measured scale-relative error ~4e-4 << 2e-2 budget.
  - E layout "(p c) h": partition p holds seq rows 32p..32p+31, so each
    E-tile descriptor is 8 KiB contiguous (128 descriptors/tile instead
    of 512) and the final store is a single contiguous [128,32] write
    (no PE transpose needed).
  - W k-tiles load FIRST, split across both HWDGE rings (SP+ACT) so v is
    ready ~9us; E tiles then stream on both rings at the ~360 GB/s
    per-core bus limit.
  - energies: tensor_tensor_reduce (mult+add-reduce) on DVE, bf16 in/out
    with fp32 accum (accum_out is a free_size-1 operand, exempt from the
    2-byte rule, so 2x_1p still applies).
  - NO warm-up collective: in this runtime each collective costs ~14.7us
    regardless of position and they serialize, so a dummy first
    collective only delays the real one (v1 trace: Comms slices
    83.2-97.9us and 99.7-114.4us back to back).
  - Distributed softmax: local (max, sum) -> one 64 B AllGather ->
    scalar combine on partition 0 -> broadcast one scale factor.
"""

import os

import numpy as np

os.environ.setdefault("BASS_PERFETTO_PROFILE_ALL_CORES", "1")

HIDDEN = 1024
SEQ = 32768
N_CORES = 8
S_SHARD = SEQ // N_CORES       # 4096 rows of E per core
P = 128
KT = HIDDEN // P               # 8 k-tiles of W
N_ETILES = 8                   # E tiles per core
SEG = S_SHARD // (P * N_ETILES)  # 4 energy columns per tile
NCOLS = N_ETILES * SEG         # 32 energy columns in SBUF

_CACHE = {}


def _build():
    import concourse.mybir as mybir
    import concourse.tile as tile
    from concourse import bacc
    from concourse import bass_isa
    from concourse.masks import make_identity

    f32 = mybir.dt.float32
    bf16 = mybir.dt.bfloat16
    Alu = mybir.AluOpType
    Act = mybir.ActivationFunctionType
    Ax = mybir.AxisListType
    Red = bass_isa.ReduceOp

    nc = bacc.Bacc("TRN2", target_bir_lowering=False, debug=False,
                   num_devices=N_CORES)

    e_d = nc.dram_tensor("e", [S_SHARD, HIDDEN], bf16, kind="ExternalInput").ap()
    w_d = nc.dram_tensor("w", [HIDDEN, HIDDEN], bf16, kind="ExternalInput").ap()
    # h[j, k] = hidden[128*k + j] (host pre-transposed)
    h_d = nc.dram_tensor("h", [P, KT], f32, kind="ExternalInput").ap()
    o_d = nc.dram_tensor("attn", [S_SHARD], f32, kind="ExternalOutput").ap()

    rg = [list(range(N_CORES))]

    with tile.TileContext(nc) as tc:
        with (
            tc.tile_pool(name="epool", bufs=N_ETILES) as epool,
            tc.tile_pool(name="wpool", bufs=KT) as wpool,
            tc.tile_pool(name="small", bufs=1) as small,
            tc.tile_pool(name="psum", bufs=1, space="PSUM") as psum,
            tc.tile_pool(name="dram", bufs=1, space="DRAM") as dram,
        ):
            # ---------------- constants ----------------
            ones128 = small.tile([P, P], bf16, tag="ones128")
            nc.vector.memset(ones128[:], 1.0)

            # Warm the ACT exp table early (~1.3us, overlaps the DMA stream).
            dummy = small.tile([1, 1], f32, tag="dummy")
            nc.vector.memset(dummy[:], 0.0)
            nc.scalar.activation(dummy[:], dummy[:], Act.Exp)

            # ---------------- W/h loads: both rings, W first -----------
            h_sb = small.tile([P, KT], f32, tag="h_sb")
            nc.sync.dma_start(h_sb[:], h_d[:])
            w_sb = []
            for k in range(KT):
                wt = wpool.tile([P, HIDDEN], bf16, tag="w")
                ring = nc.sync if k < KT // 2 else nc.scalar
                ring.dma_start(wt[:], w_d[k * P:(k + 1) * P, :])
                w_sb.append(wt)

            # ---------------- v = W.T @ h (local, replicated) -----------
            # Pre-scale W rows by h (DVE bf16 2x) so every matmul uses the
            # constant all-ones stationary: one LDWEIGHTS total and 16
            # accumulating matmuls; the result lands replicated across all
            # 128 partitions of PSUM.
            for k in range(KT):
                nc.vector.tensor_scalar_mul(w_sb[k][:], w_sb[k][:],
                                            h_sb[:, k:k + 1])
            pvb = psum.tile([P, HIDDEN], f32, tag="pvb")
            for k in range(KT):
                for n in range(2):
                    nc.tensor.matmul(pvb[:, n * 512:(n + 1) * 512],
                                     lhsT=ones128[:],
                                     rhs=w_sb[k][:, n * 512:(n + 1) * 512],
                                     start=(k == 0), stop=(k == KT - 1))
            v_sb = small.tile([P, HIDDEN], bf16, tag="v_sb")
            for n in range(2):  # bank-aligned PSUM reads, fp32 -> bf16
                nc.vector.tensor_copy(v_sb[:, n * 512:(n + 1) * 512],
                                      pvb[:, n * 512:(n + 1) * 512])

            # ---------------- energies = E @ v ----------------
            # Partition p, tile t, seg s holds E row 32p + 4t + s: energy
            # column c = 4t + s, sequence index 32p + c.
            e_view = e_d.rearrange("(p t s) h -> t p s h",
                                   p=P, t=N_ETILES, s=SEG)
            energies = small.tile([P, NCOLS], f32, tag="energies")
            scratch = small.tile([P, HIDDEN], bf16, tag="scratch")
            for t in range(N_ETILES):
                et = epool.tile([P, SEG, HIDDEN], bf16, tag="et")
                # Alternate rings; each 1 MiB tile is 128 x 8 KiB
                # descriptors.
                ring = nc.sync if t % 2 == 0 else nc.scalar
                ring.dma_start(et[:], e_view[t])
                for s in range(SEG):
                    c = t * SEG + s
                    nc.vector.affine_mul_reduce(
                        out=scratch[:],
                        accum_out=energies[:, c:c + 1],
                        in0=et[:, s],
                        in1=v_sb[:],
                        scale=1.0,
                        bias=0.0,
                    )

            # ---------------- local softmax stats ----------------
            rowmax = small.tile([P, 1], f32, tag="rowmax")
            nc.vector.reduce_max(rowmax[:], energies[:], axis=Ax.X)
            m_all = small.tile([P, 1], f32, tag="m_all")
            nc.gpsimd.partition_all_reduce(m_all[:], rowmax[:], P, Red.max)
            negm_b = small.tile([P, 1], f32, tag="negm_b")
            nc.vector.tensor_scalar_mul(negm_b[:], m_all[:], -1.0)

            ex = small.tile([P, NCOLS], f32, tag="ex")
            rowsum = small.tile([P, 1], f32, tag="rowsum")
            nc.scalar.activation(ex[:], energies[:], Act.Exp,
                                 bias=negm_b[:], scale=1.0,
                                 accum_out=rowsum[:])
            s_all = small.tile([P, 1], f32, tag="s_all")
            nc.gpsimd.partition_all_reduce(s_all[:], rowsum[:], P, Red.add)

            stats = small.tile([1, 2], f32, tag="stats")
            nc.vector.tensor_copy(stats[:, 0:1], m_all[0:1, :])
            nc.vector.tensor_copy(stats[:, 1:2], s_all[0:1, :])

            # Small mid/tail DMAs ride SWDGE (gpsimd) so they never queue
            # behind E-tile descriptors on the HWDGE rings.
            cc_s_in = dram.tile([1, 2], f32)
            cc_s_out = dram.tile([N_CORES, 2], f32)
            nc.gpsimd.dma_start(cc_s_in[:], stats[:])
            nc.gpsimd.collective_compute(
                "AllGather", Alu.bypass, replica_groups=rg,
                ins=[cc_s_in[:].opt()], outs=[cc_s_out[:].opt()],
            )
            allst = small.tile([1, 2 * N_CORES], f32, tag="allst")
            nc.gpsimd.dma_start(allst[:],
                                cc_s_out[:].rearrange("r c -> (r c)")[None])

            # ---------------- global combine (partition 0) --------------
            m_vec = allst[:].rearrange("one (r c) -> one r c", c=2)[:, :, 0]
            s_vec = allst[:].rearrange("one (r c) -> one r c", c=2)[:, :, 1]
            Mg = small.tile([1, 1], f32, tag="Mg")
            nc.vector.reduce_max(Mg[:], m_vec, axis=Ax.X)
            d = small.tile([1, N_CORES], f32, tag="d")
            nc.vector.tensor_scalar(d[:], m_vec, Mg[:], None, op0=Alu.subtract)
            edv = small.tile([1, N_CORES], f32, tag="edv")
            nc.scalar.activation(edv[:], d[:], Act.Exp)
            wsum = small.tile([1, N_CORES], f32, tag="wsum")
            Sg = small.tile([1, 1], f32, tag="Sg")
            nc.vector.affine_mul_reduce(
                out=wsum[:], accum_out=Sg[:], in0=edv[:], in1=s_vec,
                scale=1.0, bias=0.0)
            # c0 = exp(m_loc - M) / Sg  (one scalar per core)
            dm = small.tile([1, 1], f32, tag="dm")
            nc.vector.tensor_tensor(dm[:], stats[:, 0:1], Mg[:],
                                    op=Alu.subtract)
            edm = small.tile([1, 1], f32, tag="edm")
            nc.scalar.activation(edm[:], dm[:], Act.Exp)
            rS = small.tile([1, 1], f32, tag="rS")
            nc.vector.reciprocal(rS[:], Sg[:])
            c0 = small.tile([1, 1], f32, tag="c0")
            nc.vector.tensor_tensor(c0[:], edm[:], rS[:], op=Alu.mult)
            c0_b = small.tile([P, 1], f32, tag="c0_b")
            nc.gpsimd.partition_broadcast(c0_b[:], c0[:], P)

            attn = small.tile([P, NCOLS], f32, tag="attn")
            nc.vector.tensor_scalar(attn[:], ex[:], c0_b[:], None,
                                    op0=Alu.mult)
            # out[32p + c] = attn[p, c]: one contiguous [128, 32] store.
            nc.sync.dma_start(o_d.rearrange("(p c) -> p c", c=NCOLS), attn[:])

    nc.compile()
    return nc


def _get_nc():
    if "nc" not in _CACHE:
        _CACHE["nc"] = _build()
    return _CACHE["nc"]


def _in_maps(hidden, E, W):
    import ml_dtypes

    h_t = np.ascontiguousarray(hidden.reshape(KT, P).T)
    W_b = W.astype(ml_dtypes.bfloat16)
    E_b = E.astype(ml_dtypes.bfloat16)
    maps = []
    for i in range(N_CORES):
        maps.append({
            "e": np.ascontiguousarray(E_b[i * S_SHARD:(i + 1) * S_SHARD]),
            "w": W_b,
            "h": h_t,
        })
    return maps


def kernel(hidden, encoder_outputs, W, b):
    from concourse import bass_utils

    hidden = np.asarray(hidden, dtype=np.float32)
    E = np.ascontiguousarray(np.asarray(encoder_outputs, dtype=np.float32))
    W = np.asarray(W, dtype=np.float32)

    nc = _get_nc()
    res = bass_utils.run_bass_kernel_spmd(
        nc, _in_maps(hidden, E, W), core_ids=list(range(N_CORES)))
    attn = np.concatenate([res.results[i]["attn"] for i in range(N_CORES)])
    return attn.reshape(1, 1, SEQ).astype(np.float32)


# revision 12
# speedup vs baseline: 1.2416x; 1.2037x over previous
"""Trainium2 Bass kernel for nn_Attention_5669356830982.

Computes attn = softmax((E @ W.T + b) @ h)[None, None, :] for
E:[32768,1024], W:[1024,1024], h:[1024], b:[1024] (all fp32 in / fp32 out).

Algebraic rewrite: (E @ W.T + b) @ h == E @ (W.T @ h) + (b @ h); the scalar
(b @ h) shift cancels inside softmax.  So the kernel computes v = W.T @ h
(tiny) and energies = E @ v (memory-bound GEMV), then a distributed softmax.

v2 design (from the v1 trace: 132.6us, no engine >38% busy):
  - E and W are converted to bf16 on the host: halves the DMA stream
    (10.5 MiB/core total) and enables the DVE 2x_1p fast mode for the
    energy dot products.  Numerics: softmax is highly peaked (top-2
    energy gap ~5); measured scale-relative error ~4e-4 << 2e-2 budget.
  - E layout "(p c) h": partition p holds seq rows 32p..32p+31, so each
    E-tile descriptor is 8 KiB contiguous (128 descriptors/tile instead
    of 512) and the final store is a single contiguous [128,32] write
    (no PE transpose needed).
  - W k-tiles load FIRST, split across both HWDGE rings (SP+ACT) so v is
    ready ~9us; E tiles then stream on both rings at the ~360 GB/s
    per-core bus limit.
  - energies: tensor_tensor_reduce (mult+add-reduce) on DVE, bf16 in/out
    with fp32 accum (accum_out is a free_size-1 operand, exempt from the
    2-byte rule, so 2x_1p still applies).
  - NO warm-up collective: in this runtime each collective costs ~14.7us
    regardless of position and they serialize, so a dummy first
    collective only delays the real one (v1 trace: Comms slices
    83.2-97.9us and 99.7-114.4us back to back).
  - Distributed softmax: local (max, sum) -> one 64 B AllGather ->
    scalar combine on partition 0 -> broadcast one scale factor.
"""

import os

import numpy as np

os.environ.setdefault("BASS_PERFETTO_PROFILE_ALL_CORES", "1")

HIDDEN = 1024
SEQ = 32768
N_CORES = 8
S_SHARD = SEQ // N_CORES       # 4096 rows of E per core
P = 128
KT = HIDDEN // P               # 8 k-tiles of W
N_ETILES = 8                   # E tiles per core
SEG = S_SHARD // (P * N_ETILES)  # 4 energy columns per tile
NCOLS = N_ETILES * SEG         # 32 energy columns in SBUF

_CACHE = {}


def _build():
    import concourse.mybir as mybir
    import concourse.tile as tile
    from concourse import bacc
    from concourse import bass_isa
    from concourse.masks import make_identity

    f32 = mybir.dt.float32
    bf16 = mybir.dt.bfloat16
    Alu = mybir.AluOpType
    Act = mybir.ActivationFunctionType
    Ax = mybir.AxisListType
    Red = bass_isa.ReduceOp

    nc = bacc.Bacc("TRN2", target_bir_lowering=False, debug=False,
                   num_devices=N_CORES)

    e_d = nc.dram_tensor("e", [S_SHARD, HIDDEN], bf16, kind="ExternalInput").ap()
    w_d = nc.dram_tensor("w", [HIDDEN, HIDDEN], bf16, kind="ExternalInput").ap()
    # h[j, k] = hidden[128*k + j] (host pre-transposed)
    h_d = nc.dram_tensor("h", [P, KT], f32, kind="ExternalInput").ap()
    o_d = nc.dram_tensor("attn", [S_SHARD], f32, kind="ExternalOutput").ap()

    rg = [list(range(N_CORES))]

    with tile.TileContext(nc) as tc:
        with (
            tc.tile_pool(name="epool", bufs=N_ETILES) as epool,
            tc.tile_pool(name="wpool", bufs=KT) as wpool,
            tc.tile_pool(name="small", bufs=1) as small,
            tc.tile_pool(name="psum", bufs=1, space="PSUM") as psum,
            tc.tile_pool(name="dram", bufs=1, space="DRAM") as dram,
        ):
            # ---- warm-up collective: the first collective of an execution
            # cannot complete before ~98us in this runtime regardless of
            # issue time; firing a dummy AllGather at t~0 absorbs that wall
            # so the real stats AllGather at the tail runs ~15us.
            cc_w_in = dram.tile([1, 2], f32)
            cc_w_out = dram.tile([N_CORES, 2], f32)
            wz = small.tile([1, 2], f32, tag="wz")
            nc.vector.memset(wz[:], 0.0)
            nc.gpsimd.dma_start(cc_w_in[:], wz[:])
            nc.gpsimd.collective_compute(
                "AllGather", Alu.bypass, replica_groups=rg,
                ins=[cc_w_in[:].opt()], outs=[cc_w_out[:].opt()],
            )

            # ---------------- constants ----------------
            ones128 = small.tile([P, P], bf16, tag="ones128")
            nc.vector.memset(ones128[:], 1.0)

            # Warm the ACT exp table early (~1.3us, overlaps the DMA stream).
            dummy = small.tile([1, 1], f32, tag="dummy")
            nc.vector.memset(dummy[:], 0.0)
            nc.scalar.activation(dummy[:], dummy[:], Act.Exp)

            # ---------------- W/h loads: both rings, W first -----------
            h_sb = small.tile([P, KT], f32, tag="h_sb")
            nc.sync.dma_start(h_sb[:], h_d[:])
            w_sb = []
            for k in range(KT):
                wt = wpool.tile([P, HIDDEN], bf16, tag="w")
                ring = nc.sync if k < KT // 2 else nc.scalar
                ring.dma_start(wt[:], w_d[k * P:(k + 1) * P, :])
                w_sb.append(wt)

            # ---------------- v = W.T @ h (local, replicated) -----------
            # Pre-scale W rows by h (DVE bf16 2x) so every matmul uses the
            # constant all-ones stationary: one LDWEIGHTS total and 16
            # accumulating matmuls; the result lands replicated across all
            # 128 partitions of PSUM.
            for k in range(KT):
                nc.vector.tensor_scalar_mul(w_sb[k][:], w_sb[k][:],
                                            h_sb[:, k:k + 1])
            pvb = psum.tile([P, HIDDEN], f32, tag="pvb")
            for k in range(KT):
                for n in range(2):
                    nc.tensor.matmul(pvb[:, n * 512:(n + 1) * 512],
                                     lhsT=ones128[:],
                                     rhs=w_sb[k][:, n * 512:(n + 1) * 512],
                                     start=(k == 0), stop=(k == KT - 1))
            v_sb = small.tile([P, HIDDEN], bf16, tag="v_sb")
            for n in range(2):  # bank-aligned PSUM reads, fp32 -> bf16
                nc.vector.tensor_copy(v_sb[:, n * 512:(n + 1) * 512],
                                      pvb[:, n * 512:(n + 1) * 512])

            # ---------------- energies = E @ v ----------------
            # Partition p, tile t, seg s holds E row 32p + 4t + s: energy
            # column c = 4t + s, sequence index 32p + c.
            e_view = e_d.rearrange("(p t s) h -> t p s h",
                                   p=P, t=N_ETILES, s=SEG)
            energies = small.tile([P, NCOLS], f32, tag="energies")
            scratch = small.tile([P, HIDDEN], bf16, tag="scratch")
            for t in range(N_ETILES):
                et = epool.tile([P, SEG, HIDDEN], bf16, tag="et")
                # Alternate rings; each 1 MiB tile is 128 x 8 KiB
                # descriptors.
                ring = nc.sync if t % 2 == 0 else nc.scalar
                ring.dma_start(et[:], e_view[t])
                for s in range(SEG):
                    c = t * SEG + s
                    nc.vector.affine_mul_reduce(
                        out=scratch[:],
                        accum_out=energies[:, c:c + 1],
                        in0=et[:, s],
                        in1=v_sb[:],
                        scale=1.0,
                        bias=0.0,
                    )

            # ---------------- local softmax stats ----------------
            rowmax = small.tile([P, 1], f32, tag="rowmax")
            nc.vector.reduce_max(rowmax[:], energies[:], axis=Ax.X)
            m_all = small.tile([P, 1], f32, tag="m_all")
            nc.gpsimd.partition_all_reduce(m_all[:], rowmax[:], P, Red.max)
            negm_b = small.tile([P, 1], f32, tag="negm_b")
            nc.vector.tensor_scalar_mul(negm_b[:], m_all[:], -1.0)

            ex = small.tile([P, NCOLS], f32, tag="ex")
            rowsum = small.tile([P, 1], f32, tag="rowsum")
            nc.scalar.activation(ex[:], energies[:], Act.Exp,
                                 bias=negm_b[:], scale=1.0,
                                 accum_out=rowsum[:])
            s_all = small.tile([P, 1], f32, tag="s_all")
            nc.gpsimd.partition_all_reduce(s_all[:], rowsum[:], P, Red.add)

            # stats = (m, z = m + ln s): post-collective, the global
            # denominator folds into ONE exp-accumulate:
            #   S*exp(M) = sum_i exp(z_i),  via activation(Exp, bias=-Mg).
            lns = small.tile([1, 1], f32, tag="lns")
            nc.scalar.activation(lns[:], s_all[0:1, :], Act.Ln)
            z = small.tile([1, 1], f32, tag="z")
            nc.vector.tensor_tensor(z[:], m_all[0:1, :], lns[:], op=Alu.add)

            stats = small.tile([1, 2], f32, tag="stats")
            nc.vector.tensor_copy(stats[:, 0:1], m_all[0:1, :])
            nc.vector.tensor_copy(stats[:, 1:2], z[:])

            # Small mid/tail DMAs ride SWDGE (gpsimd) so they never queue
            # behind E-tile descriptors on the HWDGE rings.
            cc_s_in = dram.tile([1, 2], f32)
            cc_s_out = dram.tile([N_CORES, 2], f32)
            nc.gpsimd.dma_start(cc_s_in[:], stats[:])
            nc.gpsimd.collective_compute(
                "AllGather", Alu.bypass, replica_groups=rg,
                ins=[cc_s_in[:].opt()], outs=[cc_s_out[:].opt()],
            )
            allst = small.tile([1, 2 * N_CORES], f32, tag="allst")
            nc.gpsimd.dma_start(allst[:],
                                cc_s_out[:].rearrange("r c -> (r c)")[None])

            # ---------------- global combine (partition 0) --------------
            # S' = sum_i exp(z_i - Mg) = sum_i s_i exp(m_i - Mg): one
            # exp-accumulate replaces the subtract/exp/dot chain.
            m_vec = allst[:].rearrange("one (r c) -> one r c", c=2)[:, :, 0]
            z_vec = allst[:].rearrange("one (r c) -> one r c", c=2)[:, :, 1]
            Mg = small.tile([1, 1], f32, tag="Mg")
            nc.vector.reduce_max(Mg[:], m_vec, axis=Ax.X)
            negMg = small.tile([1, 1], f32, tag="negMg")
            nc.vector.tensor_scalar_mul(negMg[:], Mg[:], -1.0)
            ez = small.tile([1, N_CORES], f32, tag="ez")
            Sg = small.tile([1, 1], f32, tag="Sg")
            nc.scalar.activation(ez[:], z_vec, Act.Exp,
                                 bias=negMg[:], scale=1.0, accum_out=Sg[:])
            # c0 = exp(m_loc - M) / S'  (one scalar per core)
            dm = small.tile([1, 1], f32, tag="dm")
            nc.vector.tensor_tensor(dm[:], stats[:, 0:1], Mg[:],
                                    op=Alu.subtract)
            edm = small.tile([1, 1], f32, tag="edm")
            nc.scalar.activation(edm[:], dm[:], Act.Exp)
            rS = small.tile([1, 1], f32, tag="rS")
            nc.vector.reciprocal(rS[:], Sg[:])
            c0 = small.tile([1, 1], f32, tag="c0")
            nc.vector.tensor_tensor(c0[:], edm[:], rS[:], op=Alu.mult)
            c0_b = small.tile([P, 1], f32, tag="c0_b")
            nc.gpsimd.partition_broadcast(c0_b[:], c0[:], P)

            attn = small.tile([P, NCOLS], f32, tag="attn")
            nc.vector.tensor_scalar(attn[:], ex[:], c0_b[:], None,
                                    op0=Alu.mult)
            # out[32p + c] = attn[p, c]: one contiguous [128, 32] store.
            nc.sync.dma_start(o_d.rearrange("(p c) -> p c", c=NCOLS), attn[:])

    nc.compile()
    return nc


def _get_nc():
    if "nc" not in _CACHE:
        _CACHE["nc"] = _build()
    return _CACHE["nc"]


def _in_maps(hidden, E, W):
    import ml_dtypes

    h_t = np.ascontiguousarray(hidden.reshape(KT, P).T)
    W_b = W.astype(ml_dtypes.bfloat16)
    E_b = E.astype(ml_dtypes.bfloat16)
    maps = []
    for i in range(N_CORES):
        maps.append({
            "e": np.ascontiguousarray(E_b[i * S_SHARD:(i + 1) * S_SHARD]),
            "w": W_b,
            "h": h_t,
        })
    return maps


def kernel(hidden, encoder_outputs, W, b):
    from concourse import bass_utils

    hidden = np.asarray(hidden, dtype=np.float32)
    E = np.ascontiguousarray(np.asarray(encoder_outputs, dtype=np.float32))
    W = np.asarray(W, dtype=np.float32)

    nc = _get_nc()
    res = bass_utils.run_bass_kernel_spmd(
        nc, _in_maps(hidden, E, W), core_ids=list(range(N_CORES)))
    attn = np.concatenate([res.results[i]["attn"] for i in range(N_CORES)])
    return attn.reshape(1, 1, SEQ).astype(np.float32)
